# revision 1
# baseline (speedup 1.0000x reference)
"""Causal attention + output projection on 8 Trainium2 NeuronCores.

Problem (hardcoded): B=2, H=12, T=2048, D=64, DIM=768, fp32.

Sharding: 24 (b, h) pairs -> 3 heads per core; cores 0-3 take b=0,
cores 4-7 take b=1.  Each core computes attention for its 3 heads plus
the partial output projection  sum_h y_h @ W[h*64:(h+1)*64, :]  as a
(T, DIM) partial; the host sums the 4 partials per batch.  No
collectives.

Device-side layout is fully transposed ([s, q]) so no on-chip
transposes are needed:
  - host feeds qT = q^T / sqrt(D) and kT = k^T packed in one tensor
  - host feeds biasT = bias^T with the causal mask pre-added
    (-1e4 on s > q) in bf16 (halves the dominant HBM traffic)
  - v is fed augmented with 64 ones-columns so a single PV matmul
    yields both y^T (rows 0:64) and the softmax denominators
    replicated across rows 64:128.

Per (head, q-chunk of 512, group of 4 s-tiles):
  PSUM[s=128, q=2048] <- identity-matmul copy of biasT (bf16)
  PSUM                += kT-tile.T @ qT-chunk   (fp32, causally trimmed)
  SBUF P = exp(PSUM)                            (one ACT instruction)
  PSUM_y[128, 512]    += vaug-tile.T @ P-slice  (accumulated over s)
then  rec = 1/sums  (DVE, partition-realigning read 64:128 -> 0:64),
      yT[:, chunk] = y_un * rec.
Projection: out[t-block, :] accumulates yT_h-slice.T @ W_h over heads.

Build notes: the program is built on bacc.Bacc and finalize()d —
Bacc.compile()'s generate_event_semaphores pass legalizes multi-wait
instructions for this walrus build (each hardware instruction carries
at most one semaphore wait).  The head loop is a hardware `For_i`
whose back-edge barrier resets all semaphores, so every SBUF slot is
written exactly once per iteration (fresh bias buffer per head) and no
refill DMA carries a slot-release wait.  The exp is issued per PSUM
bank so subtile releases let the next group's matmuls re-enter each
bank as soon as its slice is drained (~9% on the modeled timeline).
"""

import math

import numpy as np
import ml_dtypes

B, H, T, D = 2, 12, 2048, 64
DIM = H * D
NCORES = 8
HPC = 3           # heads per core
P = 128
QC = 512          # q-chunk width (one PSUM bank of fp32)
NJ = T // QC      # 4 q-chunks
NT = T // P       # 16 s-tiles
GROUP = 4         # s-tiles per PSUM logits group (4 banks)

_PROGRAM = None


def _build_program():
    import concourse.bass as bass
    import concourse.mybir as mybir
    import concourse.tile as tile
    from concourse import bacc
    from contextlib import ExitStack

    dt = mybir.dt
    f32 = dt.float32
    bf16 = dt.bfloat16
    EXP = mybir.ActivationFunctionType.Exp
    ds = bass.ds

    nc = bacc.Bacc("TRN2", num_devices=NCORES)
    # flat layouts so per-head slices are register-offset APs
    # per-head fused [va | qT(pad) | kT(pad)] block: one DMA per head
    comb = nc.declare_dram_parameter("comb", [HPC * P, 3 * T], f32, isOutput=False)
    biasT = nc.declare_dram_parameter("biasT", [HPC * 10 * GROUP * P, QC], bf16, isOutput=False)
    wproj = nc.declare_dram_parameter("wproj", [D, HPC * DIM], f32, isOutput=False)
    out = nc.declare_dram_parameter("out", [T, DIM], f32, isOutput=True)

    with tile.TileContext(nc) as tc, ExitStack() as ctx:
        from concourse.masks import make_identity

        const_pool = ctx.enter_context(tc.tile_pool(name="const", bufs=1))
        id_t = const_pool.tile([P, P], bf16)
        make_identity(nc, id_t[:])  # gpsimd memset+affine_select: no DMA lane

        w_pool = ctx.enter_context(tc.tile_pool(name="w", bufs=1))
        w_all = w_pool.tile([D, HPC * DIM], f32)
        nc.sync.dma_start(w_all[:], wproj[:])

        yT_pool = ctx.enter_context(tc.tile_pool(name="yT", bufs=1))
        yT_t = yT_pool.tile([D, HPC * T], f32)

        with (
            tc.tile_pool(name="head", bufs=1) as head_pool,
            tc.tile_pool(name="bias", bufs=1) as bias_pool,
            tc.tile_pool(name="pexp", bufs=2) as pexp_pool,
            tc.tile_pool(name="rec", bufs=2) as rec_pool,
            tc.tile_pool(name="psl", bufs=1, space="PSUM") as psl_pool,
            tc.tile_pool(name="psy", bufs=2, space="PSUM") as psy_pool,
        ):
            with tc.For_i(0, HPC, 1) as hreg:
                cb_t = head_pool.tile([P, 3 * T], f32)
                nc.sync.dma_start(cb_t[:], comb[ds(hreg * P, P), :])
                va_t = cb_t[:, 0:T]
                qT_t = cb_t[0:D, T : 2 * T]
                kT_t = cb_t[0:D, 2 * T : 3 * T]
                NREG = 10
                b_all = bias_pool.tile([P, NREG * GROUP * QC], bf16)
                nc.scalar.dma_start(
                    b_all[:, 0 : 3 * GROUP * QC].rearrange(
                        "p (a q) -> p a q", a=3 * GROUP
                    ),
                    biasT[
                        ds(hreg * (NREG * GROUP * P), 3 * GROUP * P), :
                    ].rearrange("(a p) q -> p a q", p=P),
                )
                nc.scalar.dma_start(
                    b_all[:, 3 * GROUP * QC :].rearrange(
                        "p (a q) -> p a q", a=7 * GROUP
                    ),
                    biasT[
                        ds(hreg * (NREG * GROUP * P) + 3 * GROUP * P,
                           7 * GROUP * P),
                        :,
                    ].rearrange("(a p) q -> p a q", p=P),
                )
                for j in range(NJ):
                    psy_t = psy_pool.tile([P, QC], f32)
                    for g in range(j + 1):
                        r = j * (j + 1) // 2 + g
                        b_t = b_all[:, r * GROUP * QC : (r + 1) * GROUP * QC]
                        psl_t = psl_pool.tile([P, GROUP * QC], f32)
                        for t in range(GROUP):
                            i = g * GROUP + t
                            # bias lands first (identity copy, clears bank)
                            nc.tensor.matmul(
                                psl_t[:, t * QC : (t + 1) * QC],
                                lhsT=id_t[:],
                                rhs=b_t[:, t * QC : (t + 1) * QC],
                                start=True,
                                stop=False,
                            )
                            # causally-trimmed QK accumulate on top
                            c0 = max(0, P * i - QC * j)
                            nc.tensor.matmul(
                                psl_t[:, t * QC + c0 : (t + 1) * QC],
                                lhsT=kT_t[:, i * P : (i + 1) * P],
                                rhs=qT_t[:, j * QC + c0 : (j + 1) * QC],
                                start=False,
                                stop=True,
                            )
                        pe_t = pexp_pool.tile([P, GROUP * QC], f32)
                        # per-bank exp: subtile release lets the next group's
                        # matmuls re-enter each PSUM bank as soon as its
                        # slice is drained, instead of after the whole group
                        for t in range(GROUP):
                            nc.scalar.activation(
                                pe_t[:, t * QC : (t + 1) * QC],
                                psl_t[:, t * QC : (t + 1) * QC],
                                EXP,
                            )
                        for t in range(GROUP):
                            i = g * GROUP + t
                            nc.tensor.matmul(
                                psy_t[:],
                                lhsT=va_t[:, i * P : (i + 1) * P],
                                rhs=pe_t[:, t * QC : (t + 1) * QC],
                                start=(i == 0),
                                stop=(i == 4 * j + 3),
                            )
                    # rows 64:128 of psy hold the softmax denominators
                    # (replicated); realign to partitions 0:64 via the DVE
                    # output crossbar while taking the reciprocal.
                    rec_t = rec_pool.tile([D, QC], f32)
                    nc.vector.reciprocal(rec_t[:], psy_t[D : 2 * D, :])
                    nc.vector.tensor_mul(
                        yT_t[:, ds(hreg * T + j * QC, QC)],
                        psy_t[0:D, :],
                        rec_t[:],
                    )

        with (
            tc.tile_pool(name="psp", bufs=2, space="PSUM") as psp_pool,
            tc.tile_pool(name="outp", bufs=1) as out_pool,
        ):
            o_big = out_pool.tile([P, NT * DIM], f32)
            for tb in range(NT):
                psp_t = psp_pool.tile([P, DIM], f32)
                for o0, ow in ((0, 512), (512, 256)):
                    for h in range(HPC):
                        nc.tensor.matmul(
                            psp_t[:, o0 : o0 + ow],
                            lhsT=yT_t[:, h * T + tb * P : h * T + (tb + 1) * P],
                            rhs=w_all[:, h * DIM + o0 : h * DIM + o0 + ow],
                            start=(h == 0),
                            stop=(h == HPC - 1),
                        )
                nc.vector.tensor_copy(
                    o_big[:, tb * DIM : (tb + 1) * DIM], psp_t[:]
                )
                if tb == NT // 2 - 1:
                    nc.sync.dma_start(
                        out[0 : T // 2, :].rearrange("(a p) o -> p a o", p=P),
                        o_big[:, 0 : (NT // 2) * DIM].rearrange(
                            "p (a o) -> p a o", a=NT // 2
                        ),
                    )
            nc.sync.dma_start(
                out[T // 2 : T, :].rearrange("(a p) o -> p a o", p=P),
                o_big[:, (NT // 2) * DIM :].rearrange(
                    "p (a o) -> p a o", a=NT // 2
                ),
            )

    nc.finalize()
    return nc


def _get_program():
    global _PROGRAM
    if _PROGRAM is None:
        _PROGRAM = _build_program()
    return _PROGRAM


def make_in_maps(q, k, v, attn_bias, W_proj):
    """Host-side sharding/layout prep: one input map per core."""
    q = np.asarray(q, dtype=np.float32)
    k = np.asarray(k, dtype=np.float32)
    v = np.asarray(v, dtype=np.float32)
    attn_bias = np.asarray(attn_bias, dtype=np.float32)
    W_proj = np.asarray(W_proj, dtype=np.float32)

    scale = 1.0 / math.sqrt(D)
    # causal mask in transposed [s, q] coords: masked where s > q
    smask = (np.arange(T)[:, None] > np.arange(T)[None, :]).astype(np.float32)
    smask *= -10000.0
    w_heads = W_proj.reshape(H, D, DIM)

    in_maps = []
    for c in range(NCORES):
        b = c // 4
        h0 = HPC * (c % 4)
        hs = slice(h0, h0 + HPC)
        cb = np.zeros((HPC, P, 3 * T), dtype=np.float32)
        # va blocks: cb[:, :, k*128:(k+1)*128] = [v-tile | ones]
        va = cb[:, :, 0:T].reshape(HPC, P, NT, P)
        va[:, :, :, :D] = v[b, hs].reshape(HPC, NT, P, D).transpose(0, 2, 1, 3)
        va[:, :, :, D:] = 1.0
        cb[:, :D, T : 2 * T] = q[b, hs].transpose(0, 2, 1) * scale
        cb[:, :D, 2 * T : 3 * T] = k[b, hs].transpose(0, 2, 1)
        biasT = attn_bias[b, hs].transpose(0, 2, 1) + smask[None]
        biasT = biasT.astype(ml_dtypes.bfloat16)
        # pack the 10 causally-needed (j-chunk, s-group) regions of each
        # head contiguously: region (j, g) = rows [g*512:(g+1)*512] of
        # column chunk j
        regions = []
        for j in range(NJ):
            for g in range(j + 1):
                regions.append(
                    biasT[:, g * GROUP * P : (g + 1) * GROUP * P,
                          j * QC : (j + 1) * QC]
                )
        biasT = np.ascontiguousarray(
            np.concatenate(regions, axis=1)
        )
        in_maps.append(
            {
                "comb": cb.reshape(HPC * P, 3 * T),
                "biasT": biasT.reshape(HPC * 10 * GROUP * P, QC),
                "wproj": np.ascontiguousarray(
                    w_heads[hs].transpose(1, 0, 2).reshape(D, HPC * DIM)
                ),
            }
        )
    return in_maps


def assemble_output(results):
    """Sum the 4 per-core partial projections for each batch."""
    out = np.zeros((B, T, DIM), dtype=np.float32)
    for c in range(NCORES):
        out[c // 4] += results[c]["out"]
    return out


def kernel(q, k, v, attn_bias, W_proj):
    from concourse.bass_utils import run_bass_kernel_spmd

    nc = _get_program()
    in_maps = make_in_maps(q, k, v, attn_bias, W_proj)
    res = run_bass_kernel_spmd(nc, in_maps, list(range(NCORES)))
    return assemble_output(res.results)



# revision 12
# speedup vs baseline: 4.2629x; 4.2629x over previous
"""Causal attention + output projection on 8 Trainium2 NeuronCores.

Problem (hardcoded): B=2, H=12, T=2048, D=64, DIM=768, fp32 in/out.

Sharding: 24 (b, h) pairs -> 3 heads per core; cores 0-3 take b=0,
cores 4-7 take b=1.  Each core computes attention for its 3 heads plus
the partial output projection; the host sums the 4 fp16 partials per
batch.  No collectives.

Key design points (driven by the CoreSim cost model, where a matmul
costs out_cols x 0.42ns x cycles_per_row with fp32 = 4 cycles/row but
bf16 = 1, and DMA is ~360 B/ns):

  - Everything on the PE runs in bf16 (4x faster than the fp32
    baseline); accumulation stays fp32 in PSUM.
  - The additive attention bias is folded in MULTIPLICATIVELY:
    exp(l + b) = exp(l) * exp(b).  The host ships expb = exp(bias^T +
    causal_mask) in bf16; on-chip the ACT engine computes exp(QK) and
    the DVE multiplies by expb (2x-mode bf16).  This removes the
    baseline's identity-matmul bias copies from the PE entirely, and
    causal masking becomes exact zeros in the product.
  - V is augmented with 64 ones-columns so a single PV matmul yields
    both y^T and the softmax denominators (matmul cost is per output
    column, so the extra partitions are free).  A single DVE divide
    produces normalized y^T while moving PSUM->SBUF.
  - Diagonal s-tiles are column-trimmed (c0) in QK, exp, mul, PV and
    in the host-packed expb stream (~15% of the causal work).
  - Projection contracts (head0, head1) in one K=128 matmul plus
    head2 in a K=64 matmul (2 instead of 3 matmuls per block), output
    staged to fp16 via the otherwise-idle GPSIMD engine.
  - Layout [s, q]: s-tile on partitions, q on columns; per (chunk j of
    512 q, pair of two 128-row s-tiles) the pipeline is
    QK (PE, 2 banks) -> exp (ACT) -> *expb (DVE) -> PV (PE, into a
    1-bank psy accumulator).  psl double-buffered (4 banks) + a shared
    4-bank pool for psy/projection PSUM = exactly 8 banks.
  - The PE instruction stream is software-pipelined one pair ahead so
    the in-order PE queue never waits on exp/mul of the same pair, and
    head h+1's DMAs prefetch during head h (all input pools bufs=2).
"""

import math

import numpy as np
import ml_dtypes

B, H, T, D = 2, 12, 2048, 64
DIM = H * D
NCORES = 8
HPC = 3           # heads per core
P = 128
QC = 512          # q-chunk width
NJ = T // QC      # 4 q-chunks
NT = T // P       # 16 s-tiles

# expb packed widths: for chunk j, tile i in [0, 4j+4): width = 512 - c0
# with c0 = max(0, 128 i - 512 j) -> per-chunk cols 2048 j + 1280.
EB_COLS_J = [2048 * j + 1280 for j in range(NJ)]
EB_COLS = sum(EB_COLS_J)  # 17408 per head

_PROGRAM = None


def _c0(i, j):
    return max(0, P * i - QC * j)


def _build_program():
    import concourse.bass as bass
    import concourse.mybir as mybir
    import concourse.tile as tile
    from concourse import bacc
    from contextlib import ExitStack

    dt = mybir.dt
    f32 = dt.float32
    bf16 = dt.bfloat16
    f16 = dt.float16
    EXP = mybir.ActivationFunctionType.Exp
    DIV = mybir.AluOpType.divide
    ds = bass.ds

    nc = bacc.Bacc("TRN2", num_devices=NCORES)

    qk_d = nc.declare_dram_parameter("qk", [D, HPC * 2 * T], bf16, isOutput=False)
    va_d = nc.declare_dram_parameter("va", [P, HPC * T], bf16, isOutput=False)
    eb_d = nc.declare_dram_parameter("eb", [P, HPC * EB_COLS], bf16, isOutput=False)
    w_d = nc.declare_dram_parameter("w", [P, 2 * DIM], bf16, isOutput=False)
    out_d = nc.declare_dram_parameter("out", [T, DIM], f16, isOutput=True)

    with tile.TileContext(nc) as tc, ExitStack() as ctx:
        w_pool = ctx.enter_context(tc.tile_pool(name="w", bufs=1))
        w_t = w_pool.tile([P, 2 * DIM], bf16)
        nc.sync.dma_start(w_t[:], w_d[:])

        # yT2: heads 0,1 stacked on partitions (d of h0 on 0:64, h1 on
        # 64:128); yT1: head 2.
        yT2_pool = ctx.enter_context(tc.tile_pool(name="yT2", bufs=1))
        yT2_t = yT2_pool.tile([P, T], bf16)
        yT1_pool = ctx.enter_context(tc.tile_pool(name="yT1", bufs=1))
        yT1_t = yT1_pool.tile([D, T], bf16)

        with (
            tc.tile_pool(name="qk", bufs=2) as qk_pool,
            tc.tile_pool(name="va", bufs=2) as va_pool,
            tc.tile_pool(name="eb0", bufs=2) as eb0_pool,
            tc.tile_pool(name="eb1", bufs=2) as eb1_pool,
            tc.tile_pool(name="eb2", bufs=2) as eb2_pool,
            tc.tile_pool(name="eb3", bufs=2) as eb3_pool,
            tc.tile_pool(name="pe", bufs=3) as pe_pool,
            tc.tile_pool(name="pr", bufs=3) as pr_pool,
            tc.tile_pool(name="den", bufs=2) as den_pool,
            tc.tile_pool(name="stage", bufs=3) as stage_pool,
            tc.tile_pool(name="psl", bufs=2, space="PSUM") as psl_pool,
            tc.tile_pool(name="psy", bufs=2, space="PSUM") as psy_pool,
            tc.tile_pool(name="psp", bufs=1, space="PSUM") as psp_pool,
        ):
            eb_pools = [eb0_pool, eb1_pool, eb2_pool, eb3_pool]

            def emit_proj(j):
                """Projection for the 4 t-blocks of q-chunk j."""
                for tb in range(4 * j, 4 * j + 4):
                    st_t = stage_pool.tile([P, DIM], f16, name="st")
                    pa_t = psp_pool.tile([P, QC], f32, name="pa")
                    pb_t = psp_pool.tile([P, QC], f32, name="pb")
                    for o0, ow, ps in ((0, QC, pa_t), (QC, DIM - QC, pb_t)):
                        nc.tensor.matmul(
                            ps[:, 0:ow],
                            lhsT=yT2_t[:, tb * P : (tb + 1) * P],
                            rhs=w_t[:, o0 : o0 + ow],
                            start=True,
                            stop=False,
                        )
                        nc.tensor.matmul(
                            ps[:, 0:ow],
                            lhsT=yT1_t[:, tb * P : (tb + 1) * P],
                            rhs=w_t[0:D, DIM + o0 : DIM + o0 + ow],
                            start=False,
                            stop=True,
                        )
                    nc.vector.tensor_copy(st_t[:, 0:QC], pa_t[:])
                    nc.vector.tensor_copy(st_t[:, QC:DIM], pb_t[:, 0 : DIM - QC])
                    nc.sync.dma_start(
                        out_d[tb * P : (tb + 1) * P, :], st_t[:]
                    )

            # flat list of (h, j, p) pairs; per chunk j there are
            # 2j+2 pairs of s-tiles.
            pairs = []
            for h in range(HPC):
                for j in range(NJ):
                    for p in range(2 * j + 2):
                        pairs.append((h, j, p))

            state = {}  # per-(h,j): psy tile, eb tile + col offsets
            pv_pending = None  # (h, j, p, psl->pr tiles info)

            def emit_pv(item):
                h, j, p, pr_t = item
                psy_t = state[(h, j)]["psy"]
                last_i = 4 * j + 3
                for t in range(2):
                    i = 2 * p + t
                    c0 = _c0(i, j)
                    nc.tensor.matmul(
                        psy_t[:, c0:QC],
                        lhsT=state[(h, "va")][:, i * P : (i + 1) * P],
                        rhs=pr_t[:, t * QC + c0 : (t + 1) * QC],
                        start=(i == 0),
                        stop=(i == last_i),
                        skip_group_check=True,
                    )
                if 2 * p + 1 == last_i:
                    # chunk complete: normalize into yT (fused
                    # PSUM->SBUF move), rows 64:128 hold denominators.
                    # DVE divide fails the walrus ISA check, so use
                    # reciprocal + multiply (only one PSUM operand each).
                    if h < 2:
                        dst = yT2_t[h * D : (h + 1) * D, j * QC : (j + 1) * QC]
                    else:
                        dst = yT1_t[:, j * QC : (j + 1) * QC]
                    den_t = den_pool.tile([D, QC], f32, name="den")
                    nc.vector.reciprocal(den_t[:], psy_t[D:P, :])
                    nc.vector.tensor_mul(dst, psy_t[0:D, :], den_t[:])
                    if h == 2:
                        if j >= 1:
                            emit_proj(j - 1)

            for idx, (h, j, p) in enumerate(pairs):
                if j == 0 and p == 0:
                    # head start: input DMAs (prefetch via bufs=2)
                    qk_t = qk_pool.tile([D, 2 * T], bf16, name="qk")
                    nc.sync.dma_start(qk_t[:], qk_d[:, ds(h * 2 * T, 2 * T)])
                    va_t = va_pool.tile([P, T], bf16, name="va")
                    nc.sync.dma_start(va_t[:], va_d[:, ds(h * T, T)])
                    state[(h, "qk")] = qk_t
                    state[(h, "va")] = va_t
                    off = 0
                    for jj in range(NJ):
                        eb_t = eb_pools[jj].tile(
                            [P, EB_COLS_J[jj]], bf16, name="eb"
                        )
                        nc.sync.dma_start(
                            eb_t[:],
                            eb_d[:, ds(h * EB_COLS + off, EB_COLS_J[jj])],
                        )
                        state[(h, jj, "eb")] = eb_t
                        off += EB_COLS_J[jj]
                if p == 0:
                    psy_t = psy_pool.tile([P, QC], f32, name="psy")
                    state[(h, j)] = {"psy": psy_t}
                    # column offsets of each tile's expb slice
                    offs = []
                    o = 0
                    for i in range(4 * j + 4):
                        offs.append(o)
                        o += QC - _c0(i, j)
                    state[(h, j)]["ebo"] = offs

                qk_t = state[(h, "qk")]
                eb_t = state[(h, j, "eb")]
                ebo = state[(h, j)]["ebo"]

                psl_t = psl_pool.tile([P, 2 * QC], f32, name="psl")
                pe_t = pe_pool.tile([P, 2 * QC], bf16, name="pe")
                pr_t = pr_pool.tile([P, 2 * QC], bf16, name="pr")

                c0s = [_c0(2 * p, j), _c0(2 * p + 1, j)]
                # QK for the two s-tiles of this pair
                for t in range(2):
                    i = 2 * p + t
                    c0 = c0s[t]
                    nc.tensor.matmul(
                        psl_t[:, t * QC + c0 : (t + 1) * QC],
                        lhsT=qk_t[:, T + i * P : T + (i + 1) * P],
                        rhs=qk_t[:, j * QC + c0 : (j + 1) * QC],
                        start=True,
                        stop=True,
                    )
                # software pipeline: PV of the previous pair goes to the
                # PE queue here, after this pair's QK.
                if pv_pending is not None:
                    emit_pv(pv_pending)
                # exp then *expb, trimmed per tile on the diagonal.
                # The SBUF-only multiplies alternate DVE/GPSIMD so the
                # DVE (which also owns all PSUM reads) isn't the
                # bottleneck.
                mul_eng = nc.vector if idx % 2 == 0 else nc.gpsimd
                if c0s == [0, 0]:
                    nc.scalar.activation(pe_t[:], psl_t[:], EXP)
                    mul_eng.tensor_mul(
                        pr_t[:],
                        pe_t[:],
                        eb_t[:, ebo[2 * p] : ebo[2 * p] + 2 * QC],
                    )
                else:
                    for t in range(2):
                        i = 2 * p + t
                        c0 = c0s[t]
                        sl = slice(t * QC + c0, (t + 1) * QC)
                        nc.scalar.activation(pe_t[:, sl], psl_t[:, sl], EXP)
                        mul_eng.tensor_mul(
                            pr_t[:, sl],
                            pe_t[:, sl],
                            eb_t[:, ebo[i] : ebo[i] + QC - c0],
                        )
                pv_pending = (h, j, p, pr_t)

            emit_pv(pv_pending)
            emit_proj(NJ - 1)

    nc.finalize()
    return nc


def _get_program():
    global _PROGRAM
    if _PROGRAM is None:
        _PROGRAM = _build_program()
    return _PROGRAM


def make_in_maps(q, k, v, attn_bias, W_proj):
    """Host-side sharding/layout prep: one input map per core."""
    q = np.asarray(q, dtype=np.float32)
    k = np.asarray(k, dtype=np.float32)
    v = np.asarray(v, dtype=np.float32)
    attn_bias = np.asarray(attn_bias, dtype=np.float32)
    W_proj = np.asarray(W_proj, dtype=np.float32)
    bf = ml_dtypes.bfloat16

    scale = 1.0 / math.sqrt(D)
    # causal mask in transposed [s, q] coords: masked where s > q
    smask = np.where(
        np.arange(T)[:, None] > np.arange(T)[None, :], -10000.0, 0.0
    ).astype(np.float32)
    w_heads = W_proj.reshape(H, D, DIM)

    in_maps = []
    for c in range(NCORES):
        b = c // 4
        h0 = HPC * (c % 4)
        hs = slice(h0, h0 + HPC)

        qk = np.zeros((HPC, D, 2 * T), dtype=bf)
        qk[:, :, 0:T] = (q[b, hs].transpose(0, 2, 1) * scale).astype(bf)
        qk[:, :, T : 2 * T] = k[b, hs].transpose(0, 2, 1).astype(bf)

        va = np.ones((HPC, P, NT, P), dtype=bf)
        va[:, :, :, :D] = (
            v[b, hs].reshape(HPC, NT, P, D).transpose(0, 2, 1, 3).astype(bf)
        )

        # expb: exp(bias^T + mask) packed per (head, chunk j, tile i)
        # with diagonal column trim [c0:512)
        eb_heads = []
        for hh in range(h0, h0 + HPC):
            E = np.exp(attn_bias[b, hh].T + smask).astype(bf)
            blocks = []
            for j in range(NJ):
                for i in range(4 * j + 4):
                    c0 = _c0(i, j)
                    blocks.append(
                        E[i * P : (i + 1) * P, j * QC + c0 : (j + 1) * QC]
                    )
            eb_heads.append(np.concatenate(blocks, axis=1))
        eb = np.concatenate(eb_heads, axis=1)

        w = np.zeros((P, 2 * DIM), dtype=bf)
        w[0:D, 0:DIM] = w_heads[h0].astype(bf)
        w[D:P, 0:DIM] = w_heads[h0 + 1].astype(bf)
        w[0:D, DIM : 2 * DIM] = w_heads[h0 + 2].astype(bf)

        in_maps.append(
            {
                "qk": np.ascontiguousarray(
                    np.concatenate([qk[i] for i in range(HPC)], axis=1)
                ),
                "va": np.ascontiguousarray(
                    np.concatenate([va[i].reshape(P, T) for i in range(HPC)], axis=1)
                ),
                "eb": np.ascontiguousarray(eb),
                "w": w,
            }
        )
    return in_maps


def assemble_output(results):
    """Sum the 4 per-core fp16 partial projections for each batch."""
    out = np.zeros((B, T, DIM), dtype=np.float32)
    for c in range(NCORES):
        out[c // 4] += results[c]["out"].astype(np.float32)
    return out


def kernel(q, k, v, attn_bias, W_proj):
    from concourse.bass_utils import run_bass_kernel_spmd

    nc = _get_program()
    in_maps = make_in_maps(q, k, v, attn_bias, W_proj)
    res = run_bass_kernel_spmd(nc, in_maps, list(range(NCORES)))
    return assemble_output(res.results)
